# revision 10
# baseline (speedup 1.0000x reference)
"""AWQ 4-bit quantized linear (group size 128) on 8 Trainium2 NeuronCores.

Column-parallel: each core owns OUT/8 = 1376 output columns. The host does
layout-only prep (slicing, int4->uint8 nibble widening with the AWQ column
permutation, padding, reshapes); all arithmetic — zero-point subtract, scale
multiply, matmul, bias — runs on device.

K-major, transpose-free design. The PE stream floor is 704 matmuls x 216 ns
= 152 us; everything else is arranged to never stall it:

  - Packed weights arrive k-major ([kp=128, group, o] per otile), which IS
    the matmul stationary layout — no on-device transposes. (The previous
    o-major design needed 44 xbar-transpose DMAs whose ~358-byte packet
    storms monopolized the 16 shared DMA engines and starved the x load.)
  - Scales/zeros vary along the free (o) axis in this layout, so the host
    pre-replicates their rows across the 128 partitions (layout-only; a
    stride-0-source broadcast DMA measures 3-4x slower than streaming the
    replicated data), and dequant is two full-otile DVE tensor_tensor
    passes: qz = q - z_bc (exact small ints in fp16), w = qz * s_bc.
  - ~28 dummy matmuls on a zeroed tile run first: lifts the HAM clock gate
    to K=8/8 (~3.4 us of sustained PE activity) and keeps the PE warm
    through the DMA-bound head.
  - Ring assignment: sync (HWDGE) carries per-otile weights + broadcasts +
    the tail stores; scalar (HWDGE) carries the resident x slabs; the slow
    gpsimd SWDGE ring only carries latency-tolerant mid-kernel out stores.
  - PE matmul accumulation over the 32 k-groups into PSUM:
      outT[o, m] += w[kp, g, o].T @ xT[kp, g, m]
    evacuated with bias via ACT (m-chunk 0) and DVE (m-chunk 1).
"""

import os
import sys

import numpy as np

if "/opt/trn_rl_repo" not in sys.path:
    sys.path.insert(0, "/opt/trn_rl_repo")

M, IN, OUT = 1024, 4096, 11008
N_CORES = 8
OC = OUT // N_CORES  # 1376 output columns per core
GS = 128  # quantization group size (== matmul k-tile)
G = IN // GS  # 32 groups
PACK = 8  # int4 values per int32 word
# reference unpacks nibble k to logical column AWQ_REVERSE_ORDER.index(k);
# equivalently logical column j within a word uses shift 4*REV[j]:
REV = np.array([0, 4, 1, 5, 2, 6, 3, 7], dtype=np.uint32)

MM_N = 512  # moving-operand free size per matmul (one PSUM bank of fp32)
N_DUMMY = 40  # PE warmup matmuls (~3.4us cold + ~7us warm coverage)
FUSE = 3  # otiles co-scheduled in the first k-sweep (paces x consumption
# at ~200 GB/s, under the ~360-440 GB/s per-core DMA fleet ceiling, while
# the resident x and the next otile preps stream in)
QC = 8  # groups per dequant chunk (pipelining granularity)

_CACHE = {}


def _unpack_int4(q: np.ndarray) -> np.ndarray:
    """[rows, cols//8] int32 -> [rows, cols] uint8 in 0..15 (AWQ order)."""
    qu = q.view(np.uint32)
    nib = (qu[:, :, None] >> (REV * 4)[None, None, :]) & 0xF
    return nib.reshape(q.shape[0], -1).astype(np.uint8)


def _build(m, k, oc, n_cores):
    import concourse.bacc as bacc
    import concourse.tile as tile
    from concourse import mybir

    F16 = mybir.dt.float16
    F32 = mybir.dt.float32
    U8 = mybir.dt.uint8
    IDENT = mybir.ActivationFunctionType.Identity

    g = k // GS
    n_otiles = (oc + 127) // 128
    n_mch = (m + MM_N - 1) // MM_N
    n_qc = g // QC

    nc = bacc.Bacc("TRN2", target_bir_lowering=False, debug=False)
    # x pre-swizzled on host to the SBUF layout [partition, group, m] so the
    # load runs with contiguous multi-KB per-partition descriptors
    x3 = nc.dram_tensor("x3", [128, g, m], F16, kind="ExternalInput").ap()
    # k-major packed weights: [otile, kp, group, o] — per-partition runs of
    # 4 KB per otile, and the dequant output needs no transpose
    qwK = nc.dram_tensor("qwK", [n_otiles, 128, g, 128], U8, kind="ExternalInput").ap()
    # host-replicated per-otile scale / zero planes (partition-major)
    sbcK = nc.dram_tensor("sbcK", [n_otiles, 128, g, 128], F16, kind="ExternalInput").ap()
    zbcK = nc.dram_tensor("zbcK", [n_otiles, 128, g, 128], U8, kind="ExternalInput").ap()
    biasT = nc.dram_tensor("biasT", [128, n_otiles, 1], F32, kind="ExternalInput").ap()
    outT = nc.dram_tensor("outT", [oc, m], F16, kind="ExternalOutput").ap()
    warm = nc.dram_tensor("warm", [128, 1], F32, kind="ExternalOutput").ap()

    with tile.TileContext(nc) as tc:
        with (
            tc.tile_pool(name="x", bufs=1) as xpool,
            tc.tile_pool(name="consts", bufs=1) as cpool,
            tc.tile_pool(name="q", bufs=4) as qpool,
            tc.tile_pool(name="sbc", bufs=4) as sbcpool,
            tc.tile_pool(name="zbc", bufs=4) as zbcpool,
            tc.tile_pool(name="qz", bufs=6) as qzpool,
            tc.tile_pool(name="w", bufs=5) as wpool,
            tc.tile_pool(name="ps", bufs=7, space="PSUM") as pspool,
            tc.tile_pool(name="wps", bufs=1, space="PSUM") as wmpool,
            tc.tile_pool(name="o", bufs=4) as opool,
        ):
            # resident transposed activations: [128, g, m]
            xT_sb = xpool.tile([128, g, m], F16)

            # PE warmup: dummy matmuls on a zeroed tile, no data deps. The
            # result is read once at the very end (warm output) so the chain
            # cannot be considered dead.
            dmy_t = cpool.tile([128, MM_N], F16, tag="dmy")
            dps_t = wmpool.tile([128, MM_N], F32, tag="wps")
            nc.gpsimd.memset(dmy_t[:], 0.0)
            for _ in range(N_DUMMY):
                nc.tensor.matmul(
                    dps_t[:], dmy_t[:, :128], dmy_t[:],
                    start=True, stop=True, skip_group_check=True,
                )

            def load_consts():
                b_t = cpool.tile([128, n_otiles, 1], F32, tag="b")
                nc.sync.dma_start(b_t[:], biasT[:])
                return b_t

            def load_x(slabs):
                for g0, g1 in slabs:
                    nc.scalar.dma_start(xT_sb[:, g0:g1], x3[:, g0:g1])

            def _deq_chunk(c, q_t, s_bc, z_bc, w_t):
                gsl = slice(c * QC, (c + 1) * QC)
                qz_t = qzpool.tile([128, QC, 128], F16, tag="qz")
                nc.vector.tensor_tensor(
                    qz_t[:], q_t[:, gsl], z_bc[:, 0, gsl],
                    mybir.AluOpType.subtract,
                )
                nc.vector.tensor_tensor(
                    w_t[:, gsl], qz_t[:], s_bc[:, 0, gsl],
                    mybir.AluOpType.mult,
                )

            def _prep_tiles():
                q_t = qpool.tile([128, g, 128], U8, tag="q")
                s_bc = sbcpool.tile([128, 1, g, 128], F16, tag="sbc")
                z_bc = zbcpool.tile([128, 1, g, 128], U8, tag="zbc")
                w_t = wpool.tile([128, g, 128], F16, tag="w")
                return q_t, s_bc, z_bc, w_t

            def prep_sweep(ots):
                # chunk-interleaved prep for the fused sweep: the sweep needs
                # chunk c of every otile only by ~c*10us, so stream rounds of
                # [chunk c of each otile] instead of whole otiles
                tiles = {ot: _prep_tiles() for ot in ots}
                for c in range(n_qc):
                    for ot in ots:
                        q_t, s_bc, z_bc, _ = tiles[ot]
                        gsl = slice(c * QC, (c + 1) * QC)
                        nc.sync.dma_start(q_t[:, gsl], qwK[ot, :, gsl])
                        nc.sync.dma_start(s_bc[:, 0, gsl], sbcK[ot, :, gsl])
                        nc.sync.dma_start(z_bc[:, 0, gsl], zbcK[ot, :, gsl])
                    for ot in ots:
                        _deq_chunk(c, *tiles[ot])
                return [tiles[ot][3] for ot in ots]

            def prep_otile(ot):
                # steady-phase prep: whole-otile DMAs, rings alternated
                q_t, s_bc, z_bc, w_t = _prep_tiles()
                eng = nc.scalar if ot % 2 else nc.sync
                eng.dma_start(q_t[:], qwK[ot])
                eng.dma_start(s_bc[:, 0], sbcK[ot])
                eng.dma_start(z_bc[:, 0], zbcK[ot])
                for c in range(n_qc):
                    _deq_chunk(c, q_t, s_bc, z_bc, w_t)
                return w_t

            mslices = [
                slice(mc * MM_N, min(m, (mc + 1) * MM_N)) for mc in range(n_mch)
            ]

            def mm_sweep(ots, w_ts, mid_hook=None):
                # co-scheduled k-sweep over several otiles: one PSUM chain
                # per (otile, m-chunk); paces per-byte x reuse
                pss = {
                    (ot, mc): pspool.tile(
                        [128, MM_N], F32, name=f"ps_{ot}_{mc}", tag="ps"
                    )
                    for ot in ots
                    for mc in range(n_mch)
                }
                for gi in range(g):
                    if mid_hook is not None and gi == g // 2:
                        mid_hook()
                    for ot, w_t in zip(ots, w_ts):
                        for mc in range(n_mch):
                            msl = mslices[mc]
                            nc.tensor.matmul(
                                pss[(ot, mc)][:, : msl.stop - msl.start],
                                w_t[:, gi, :],
                                xT_sb[:, gi, msl],
                                start=(gi == 0),
                                stop=(gi == g - 1),
                            )
                return pss

            def evac_otile(ot, pss, b_all, last):
                o0 = ot * 128
                ob = min(128, oc - o0)
                b_t = b_all[:ob, ot]
                for mc in range(n_mch):
                    msl = mslices[mc]
                    mn = msl.stop - msl.start
                    ps = pss[(ot, mc)]
                    o_t = opool.tile([128, MM_N], F16)
                    if not last:
                        # evac split across ACT and DVE to balance engine load
                        if mc == 0:
                            nc.scalar.activation(
                                o_t[:ob, :mn], ps[:ob, :mn], IDENT,
                                bias=b_t[:], scale=1.0,
                            )
                        else:
                            nc.vector.tensor_scalar_add(
                                o_t[:ob, :mn], ps[:ob, :mn], b_t[:]
                            )
                        nc.gpsimd.dma_start(outT[o0 : o0 + ob, msl], o_t[:ob, :mn])
                    else:
                        # tail: pipeline evac+store in half-chunks on the
                        # fast sync ring
                        h = mn // 2
                        for hf in range(2):
                            osl = slice(hf * h, (hf + 1) * h)
                            hsl = slice(msl.start + hf * h, msl.start + (hf + 1) * h)
                            if hf == 0:
                                nc.scalar.activation(
                                    o_t[:ob, osl], ps[:ob, osl], IDENT,
                                    bias=b_t[:], scale=1.0,
                                )
                            else:
                                nc.vector.tensor_scalar_add(
                                    o_t[:ob, osl], ps[:ob, osl], b_t[:]
                                )
                            nc.sync.dma_start(outT[o0 : o0 + ob, hsl], o_t[:ob, osl])

            AHEAD = 2
            b_all = load_consts()
            # 2-group slabs: 4 KB per-partition runs, matching the weight
            # stream's packet size so ring arbitration shares ~50:50
            load_x([(i, i + 2) for i in range(0, 8, 2)])
            prep = prep_sweep(list(range(min(FUSE, n_otiles))))
            load_x([(i, i + 2) for i in range(8, 32, 2)])
            # fused first sweep over otiles 0..FUSE-1; prep(FUSE) is injected
            # at the sweep midpoint so its DMAs stream during the sweep
            sweep_ots = list(range(min(FUSE, n_otiles)))
            pss = mm_sweep(
                sweep_ots, prep[:],
                mid_hook=(lambda: prep.append(prep_otile(FUSE)))
                if FUSE < n_otiles else None,
            )
            prep = prep[len(sweep_ots):]
            if FUSE + 1 < n_otiles:
                prep.append(prep_otile(FUSE + 1))
            for ot in sweep_ots:
                evac_otile(ot, pss, b_all, last=(ot == n_otiles - 1))
            # serial phase for the remaining otiles
            for ot in range(FUSE, n_otiles):
                if ot + AHEAD < n_otiles:
                    prep.append(prep_otile(ot + AHEAD))
                w_t = prep.pop(0)
                ps1 = mm_sweep([ot], [w_t])
                evac_otile(ot, ps1, b_all, last=(ot == n_otiles - 1))

            # consume the warmup chain at the very end so it can't be elided
            wm_t = cpool.tile([128, 1], F32, tag="wm")
            nc.vector.tensor_copy(wm_t[:], dps_t[:, :1])
            nc.gpsimd.dma_start(warm[:], wm_t[:])

    nc.compile()
    return nc


def _get_nc(m=M, k=IN, oc=OC, n_cores=N_CORES):
    key = (m, k, oc, n_cores)
    if key not in _CACHE:
        _CACHE[key] = _build(*key)
    return _CACHE[key]


def _make_in_maps(x, qweight, qzeros, scales, bias, n_cores=N_CORES):
    iw8 = _unpack_int4(qweight)  # [IN, OUT] uint8
    iz8 = _unpack_int4(qzeros)  # [G, OUT] uint8
    kk, mm = x.shape[1], x.shape[0]
    # [p, group, m]: partition-major so each partition's slab is contiguous
    x3 = np.ascontiguousarray(x.T.reshape(kk // GS, GS, mm).transpose(1, 0, 2))
    oc = qweight.shape[1] * PACK // n_cores
    n_ot = (oc + 127) // 128
    ocp = n_ot * 128
    g = kk // GS

    def padc(a):
        # pad the o (last) axis to whole otiles
        return np.pad(a, [(0, 0)] * (a.ndim - 1) + [(0, ocp - oc)])

    def pm(a):
        # pad rows to whole otiles, then [ocp, d] -> [128, n_ot, d]
        a = np.pad(a, [(0, ocp - oc)] + [(0, 0)] * (a.ndim - 1))
        return np.ascontiguousarray(a.reshape(n_ot, 128, -1).transpose(1, 0, 2))

    in_maps = []
    for c in range(n_cores):
        sl = slice(c * oc, (c + 1) * oc)
        # [k, o] -> [g, kp, n_ot, o128] -> [n_ot, kp, g, o128]
        iw = padc(iw8[:, sl]).reshape(g, GS, n_ot, 128)
        qwK = np.ascontiguousarray(iw.transpose(2, 1, 0, 3))
        s = padc(scales[:, sl]).reshape(g, n_ot, 128)
        sKot = np.ascontiguousarray(s.transpose(1, 0, 2))[:, None]  # [ot,1,g,128]
        z = padc(iz8[:, sl]).reshape(g, n_ot, 128)
        zKot = np.ascontiguousarray(z.transpose(1, 0, 2))[:, None]
        sbcK = np.ascontiguousarray(np.broadcast_to(sKot, (n_ot, 128, g, 128)))
        zbcK = np.ascontiguousarray(np.broadcast_to(zKot, (n_ot, 128, g, 128)))
        in_maps.append(
            {
                "x3": x3,
                "qwK": qwK,
                "sbcK": sbcK,
                "zbcK": zbcK,
                "biasT": pm(bias[sl].reshape(-1, 1).astype(np.float32)),
            }
        )
    return in_maps


LAST_EXEC_NS = None


def kernel(x, qweight, qzeros, scales, bias):
    global LAST_EXEC_NS
    from concourse.bass_utils import run_bass_kernel_spmd

    x = np.asarray(x)
    qweight = np.asarray(qweight)
    qzeros = np.asarray(qzeros)
    scales = np.asarray(scales)
    bias = np.asarray(bias)

    nc = _get_nc()
    in_maps = _make_in_maps(x, qweight, qzeros, scales, bias)

    kwargs = {}
    if os.environ.get("AWQ_PROFILE"):
        _enable_profiling()
        kwargs = dict(trace=True, tmpdir=os.environ.get("AWQ_TRACE_DIR") or None)
    res = run_bass_kernel_spmd(nc, in_maps, list(range(N_CORES)), **kwargs)
    LAST_EXEC_NS = res.exec_time_ns

    outT = np.concatenate([res.results[c]["outT"] for c in range(N_CORES)], axis=0)
    return np.ascontiguousarray(outT.T)


def _enable_profiling():
    """Register the NTFF profile hook missing from this image's antenv."""
    import types

    if "antenv.axon_hooks" not in sys.modules:
        import antenv

        mod = types.ModuleType("antenv.axon_hooks")
        mod._hook = None
        mod.set_axon_ntff_profile_hook = lambda h: setattr(mod, "_hook", h)
        mod.get_axon_ntff_profile_hook = lambda: mod._hook
        sys.modules["antenv.axon_hooks"] = mod
        antenv.axon_hooks = mod
        try:
            from trn_agent_boot.trn_boot import _ntff_profile_via_ctypes

            mod.set_axon_ntff_profile_hook(
                _ntff_profile_via_ctypes("/opt/axon/libaxon_pjrt.so")
            )
        except Exception:
            pass
    import concourse.bass_utils as _bu

    _bu.upload_artifacts = lambda tmpdir: "local://skipped"


# revision 12
# speedup vs baseline: 1.0855x; 1.0855x over previous
"""AWQ 4-bit quantized linear (group size 128) on 8 Trainium2 NeuronCores.

Column-parallel: each core owns OUT/8 = 1376 output columns. The host does
layout-only prep (slicing, int4->uint8 nibble widening with the AWQ column
permutation, padding, reshapes); all arithmetic — zero-point subtract, scale
multiply, matmul, bias — runs on device.

K-major, transpose-free design. The PE stream floor is 704 matmuls x 216 ns
= 152 us; everything else is arranged to never stall it:

  - Packed weights arrive k-major ([kp=128, group, o] per otile), which IS
    the matmul stationary layout — no on-device transposes. (The previous
    o-major design needed 44 xbar-transpose DMAs whose ~358-byte packet
    storms monopolized the 16 shared DMA engines and starved the x load.)
  - Scales/zeros vary along the free (o) axis in this layout, so the host
    pre-replicates their rows across the 128 partitions (layout-only; a
    stride-0-source broadcast DMA measures 3-4x slower than streaming the
    replicated data), and dequant is two full-otile DVE tensor_tensor
    passes: qz = q - z_bc (exact small ints in fp16), w = qz * s_bc.
  - ~28 dummy matmuls on a zeroed tile run first: lifts the HAM clock gate
    to K=8/8 (~3.4 us of sustained PE activity) and keeps the PE warm
    through the DMA-bound head.
  - Ring assignment: sync (HWDGE) carries per-otile weights + broadcasts +
    the tail stores; scalar (HWDGE) carries the resident x slabs; the slow
    gpsimd SWDGE ring only carries latency-tolerant mid-kernel out stores.
  - PE matmul accumulation over the 32 k-groups into PSUM:
      outT[o, m] += w[kp, g, o].T @ xT[kp, g, m]
    evacuated with bias via ACT (m-chunk 0) and DVE (m-chunk 1).
"""

import os
import sys

import numpy as np

if "/opt/trn_rl_repo" not in sys.path:
    sys.path.insert(0, "/opt/trn_rl_repo")

M, IN, OUT = 1024, 4096, 11008
N_CORES = 8
OC = OUT // N_CORES  # 1376 output columns per core
GS = 128  # quantization group size (== matmul k-tile)
G = IN // GS  # 32 groups
PACK = 8  # int4 values per int32 word
# reference unpacks nibble k to logical column AWQ_REVERSE_ORDER.index(k);
# equivalently logical column j within a word uses shift 4*REV[j]:
REV = np.array([0, 4, 1, 5, 2, 6, 3, 7], dtype=np.uint32)

MM_N = 512  # moving-operand free size per matmul (one PSUM bank of fp32)
N_DUMMY = 34  # PE warmup matmuls (~3.4us cold + ~5.6us warm coverage)
FUSE = 3  # otiles co-scheduled in the first k-sweep (paces x consumption
# at ~200 GB/s, under the ~360-440 GB/s per-core DMA fleet ceiling, while
# the resident x and the next otile preps stream in)
QC = 8  # groups per dequant chunk (pipelining granularity)

_CACHE = {}


def _unpack_int4(q: np.ndarray) -> np.ndarray:
    """[rows, cols//8] int32 -> [rows, cols] uint8 in 0..15 (AWQ order)."""
    qu = q.view(np.uint32)
    nib = (qu[:, :, None] >> (REV * 4)[None, None, :]) & 0xF
    return nib.reshape(q.shape[0], -1).astype(np.uint8)


def _build(m, k, oc, n_cores):
    import concourse.bacc as bacc
    import concourse.tile as tile
    from concourse import mybir

    F16 = mybir.dt.float16
    F32 = mybir.dt.float32
    U8 = mybir.dt.uint8
    IDENT = mybir.ActivationFunctionType.Identity

    g = k // GS
    n_otiles = (oc + 127) // 128
    n_mch = (m + MM_N - 1) // MM_N
    n_qc = g // QC

    nc = bacc.Bacc("TRN2", target_bir_lowering=False, debug=False)
    # x pre-swizzled on host to the SBUF layout [partition, group, m] so the
    # load runs with contiguous multi-KB per-partition descriptors
    x3 = nc.dram_tensor("x3", [128, g, m], F16, kind="ExternalInput").ap()
    # k-major packed weights: [otile, kp, group, o] — per-partition runs of
    # 4 KB per otile, and the dequant output needs no transpose
    qwK = nc.dram_tensor("qwK", [n_otiles, 128, g, 128], U8, kind="ExternalInput").ap()
    # host-replicated per-otile scale / zero planes (partition-major)
    sbcK = nc.dram_tensor("sbcK", [n_otiles, 128, g, 128], F16, kind="ExternalInput").ap()
    zbcK = nc.dram_tensor("zbcK", [n_otiles, 128, g, 128], U8, kind="ExternalInput").ap()
    biasT = nc.dram_tensor("biasT", [128, n_otiles, 1], F32, kind="ExternalInput").ap()
    outT = nc.dram_tensor("outT", [oc, m], F16, kind="ExternalOutput").ap()
    warm = nc.dram_tensor("warm", [128, 1], F32, kind="ExternalOutput").ap()

    with tile.TileContext(nc) as tc:
        with (
            tc.tile_pool(name="x", bufs=1) as xpool,
            tc.tile_pool(name="consts", bufs=1) as cpool,
            tc.tile_pool(name="q", bufs=4) as qpool,
            tc.tile_pool(name="sbc", bufs=4) as sbcpool,
            tc.tile_pool(name="zbc", bufs=4) as zbcpool,
            tc.tile_pool(name="qz", bufs=6) as qzpool,
            tc.tile_pool(name="w", bufs=5) as wpool,
            tc.tile_pool(name="ps", bufs=7, space="PSUM") as pspool,
            tc.tile_pool(name="wps", bufs=1, space="PSUM") as wmpool,
            tc.tile_pool(name="o", bufs=4) as opool,
        ):
            # resident transposed activations: [128, g, m]
            xT_sb = xpool.tile([128, g, m], F16)

            # PE warmup: dummy matmuls on a zeroed tile, no data deps. The
            # result is read once at the very end (warm output) so the chain
            # cannot be considered dead.
            dmy_t = cpool.tile([128, MM_N], F16, tag="dmy")
            dps_t = wmpool.tile([128, MM_N], F32, tag="wps")
            nc.gpsimd.memset(dmy_t[:], 0.0)
            for _ in range(N_DUMMY):
                nc.tensor.matmul(
                    dps_t[:], dmy_t[:, :128], dmy_t[:],
                    start=True, stop=True, skip_group_check=True,
                )

            def load_consts():
                b_t = cpool.tile([128, n_otiles, 1], F32, tag="b")
                nc.sync.dma_start(b_t[:], biasT[:])
                return b_t

            def load_x(slabs):
                for g0, g1 in slabs:
                    nc.scalar.dma_start(xT_sb[:, g0:g1], x3[:, g0:g1])

            def _deq_chunk(c, q_t, s_bc, z_bc, w_t):
                gsl = slice(c * QC, (c + 1) * QC)
                qz_t = qzpool.tile([128, QC, 128], F16, tag="qz")
                nc.vector.tensor_tensor(
                    qz_t[:], q_t[:, gsl], z_bc[:, 0, gsl],
                    mybir.AluOpType.subtract,
                )
                nc.vector.tensor_tensor(
                    w_t[:, gsl], qz_t[:], s_bc[:, 0, gsl],
                    mybir.AluOpType.mult,
                )

            def _prep_tiles():
                q_t = qpool.tile([128, g, 128], U8, tag="q")
                s_bc = sbcpool.tile([128, 1, g, 128], F16, tag="sbc")
                z_bc = zbcpool.tile([128, 1, g, 128], U8, tag="zbc")
                w_t = wpool.tile([128, g, 128], F16, tag="w")
                return q_t, s_bc, z_bc, w_t

            def prep_sweep(ots):
                # chunk-interleaved prep for the fused sweep: the sweep needs
                # chunk c of every otile only by ~c*10us, so stream rounds of
                # [chunk c of each otile] instead of whole otiles
                tiles = {ot: _prep_tiles() for ot in ots}
                for c in range(n_qc):
                    for ot in ots:
                        q_t, s_bc, z_bc, _ = tiles[ot]
                        gsl = slice(c * QC, (c + 1) * QC)
                        nc.sync.dma_start(q_t[:, gsl], qwK[ot, :, gsl])
                        nc.sync.dma_start(s_bc[:, 0, gsl], sbcK[ot, :, gsl])
                        nc.sync.dma_start(z_bc[:, 0, gsl], zbcK[ot, :, gsl])
                    for ot in ots:
                        _deq_chunk(c, *tiles[ot])
                return [tiles[ot][3] for ot in ots]

            def prep_otile(ot):
                # steady-phase prep: whole-otile DMAs on sync (the scalar
                # ring is still draining the x slabs during the sweep)
                q_t, s_bc, z_bc, w_t = _prep_tiles()
                eng = nc.sync
                eng.dma_start(q_t[:], qwK[ot])
                eng.dma_start(s_bc[:, 0], sbcK[ot])
                eng.dma_start(z_bc[:, 0], zbcK[ot])
                for c in range(n_qc):
                    _deq_chunk(c, q_t, s_bc, z_bc, w_t)
                return w_t

            mslices = [
                slice(mc * MM_N, min(m, (mc + 1) * MM_N)) for mc in range(n_mch)
            ]

            def mm_sweep(ots, w_ts, hooks=None):
                # co-scheduled k-sweep over several otiles: one PSUM chain
                # per (otile, m-chunk); paces per-byte x reuse
                pss = {
                    (ot, mc): pspool.tile(
                        [128, MM_N], F32, name=f"ps_{ot}_{mc}", tag="ps"
                    )
                    for ot in ots
                    for mc in range(n_mch)
                }
                for gi in range(g):
                    if hooks and gi in hooks:
                        hooks[gi]()
                    for ot, w_t in zip(ots, w_ts):
                        for mc in range(n_mch):
                            msl = mslices[mc]
                            nc.tensor.matmul(
                                pss[(ot, mc)][:, : msl.stop - msl.start],
                                w_t[:, gi, :],
                                xT_sb[:, gi, msl],
                                start=(gi == 0),
                                stop=(gi == g - 1),
                            )
                return pss

            def evac_otile(ot, pss, b_all, last):
                o0 = ot * 128
                ob = min(128, oc - o0)
                b_t = b_all[:ob, ot]
                for mc in range(n_mch):
                    msl = mslices[mc]
                    mn = msl.stop - msl.start
                    ps = pss[(ot, mc)]
                    o_t = opool.tile([128, MM_N], F16)
                    if not last:
                        # evac split across ACT and DVE to balance engine load
                        if mc == 0:
                            nc.scalar.activation(
                                o_t[:ob, :mn], ps[:ob, :mn], IDENT,
                                bias=b_t[:], scale=1.0,
                            )
                        else:
                            nc.vector.tensor_scalar_add(
                                o_t[:ob, :mn], ps[:ob, :mn], b_t[:]
                            )
                        nc.gpsimd.dma_start(outT[o0 : o0 + ob, msl], o_t[:ob, :mn])
                    else:
                        # tail: pipeline evac+store in half-chunks on the
                        # fast sync ring
                        h = mn // 2
                        for hf in range(2):
                            osl = slice(hf * h, (hf + 1) * h)
                            hsl = slice(msl.start + hf * h, msl.start + (hf + 1) * h)
                            if hf == 0:
                                nc.scalar.activation(
                                    o_t[:ob, osl], ps[:ob, osl], IDENT,
                                    bias=b_t[:], scale=1.0,
                                )
                            else:
                                nc.vector.tensor_scalar_add(
                                    o_t[:ob, osl], ps[:ob, osl], b_t[:]
                                )
                            nc.sync.dma_start(outT[o0 : o0 + ob, hsl], o_t[:ob, osl])

            AHEAD = 3
            b_all = load_consts()
            # 2-group slabs: 4 KB per-partition runs, matching the weight
            # stream's packet size so ring arbitration shares ~50:50
            load_x([(i, i + 2) for i in range(0, 8, 2)])
            prep = prep_sweep(list(range(min(FUSE, n_otiles))))
            load_x([(i, i + 2) for i in range(8, 32, 2)])
            # fused first sweep over otiles 0..FUSE-1; prep(FUSE) is injected
            # at the sweep midpoint so its DMAs stream during the sweep
            sweep_ots = list(range(min(FUSE, n_otiles)))
            hooks = {}
            if FUSE < n_otiles:
                hooks[12] = lambda: prep.append(prep_otile(FUSE))
            if FUSE + 1 < n_otiles:
                hooks[22] = lambda: prep.append(prep_otile(FUSE + 1))
            pss = mm_sweep(sweep_ots, prep[:], hooks=hooks)
            prep = prep[len(sweep_ots):]
            if FUSE + 2 < n_otiles:
                prep.append(prep_otile(FUSE + 2))
            for ot in sweep_ots:
                evac_otile(ot, pss, b_all, last=(ot == n_otiles - 1))
            # serial phase for the remaining otiles
            for ot in range(FUSE, n_otiles):
                if ot + AHEAD < n_otiles:
                    prep.append(prep_otile(ot + AHEAD))
                w_t = prep.pop(0)
                ps1 = mm_sweep([ot], [w_t])
                evac_otile(ot, ps1, b_all, last=(ot == n_otiles - 1))

            # consume the warmup chain at the very end so it can't be elided
            wm_t = cpool.tile([128, 1], F32, tag="wm")
            nc.vector.tensor_copy(wm_t[:], dps_t[:, :1])
            nc.gpsimd.dma_start(warm[:], wm_t[:])

    nc.compile()
    return nc


def _get_nc(m=M, k=IN, oc=OC, n_cores=N_CORES):
    key = (m, k, oc, n_cores)
    if key not in _CACHE:
        _CACHE[key] = _build(*key)
    return _CACHE[key]


def _make_in_maps(x, qweight, qzeros, scales, bias, n_cores=N_CORES):
    iw8 = _unpack_int4(qweight)  # [IN, OUT] uint8
    iz8 = _unpack_int4(qzeros)  # [G, OUT] uint8
    kk, mm = x.shape[1], x.shape[0]
    # [p, group, m]: partition-major so each partition's slab is contiguous
    x3 = np.ascontiguousarray(x.T.reshape(kk // GS, GS, mm).transpose(1, 0, 2))
    oc = qweight.shape[1] * PACK // n_cores
    n_ot = (oc + 127) // 128
    ocp = n_ot * 128
    g = kk // GS

    def padc(a):
        # pad the o (last) axis to whole otiles
        return np.pad(a, [(0, 0)] * (a.ndim - 1) + [(0, ocp - oc)])

    def pm(a):
        # pad rows to whole otiles, then [ocp, d] -> [128, n_ot, d]
        a = np.pad(a, [(0, ocp - oc)] + [(0, 0)] * (a.ndim - 1))
        return np.ascontiguousarray(a.reshape(n_ot, 128, -1).transpose(1, 0, 2))

    in_maps = []
    for c in range(n_cores):
        sl = slice(c * oc, (c + 1) * oc)
        # [k, o] -> [g, kp, n_ot, o128] -> [n_ot, kp, g, o128]
        iw = padc(iw8[:, sl]).reshape(g, GS, n_ot, 128)
        qwK = np.ascontiguousarray(iw.transpose(2, 1, 0, 3))
        s = padc(scales[:, sl]).reshape(g, n_ot, 128)
        sKot = np.ascontiguousarray(s.transpose(1, 0, 2))[:, None]  # [ot,1,g,128]
        z = padc(iz8[:, sl]).reshape(g, n_ot, 128)
        zKot = np.ascontiguousarray(z.transpose(1, 0, 2))[:, None]
        sbcK = np.ascontiguousarray(np.broadcast_to(sKot, (n_ot, 128, g, 128)))
        zbcK = np.ascontiguousarray(np.broadcast_to(zKot, (n_ot, 128, g, 128)))
        in_maps.append(
            {
                "x3": x3,
                "qwK": qwK,
                "sbcK": sbcK,
                "zbcK": zbcK,
                "biasT": pm(bias[sl].reshape(-1, 1).astype(np.float32)),
            }
        )
    return in_maps


LAST_EXEC_NS = None


def kernel(x, qweight, qzeros, scales, bias):
    global LAST_EXEC_NS
    from concourse.bass_utils import run_bass_kernel_spmd

    x = np.asarray(x)
    qweight = np.asarray(qweight)
    qzeros = np.asarray(qzeros)
    scales = np.asarray(scales)
    bias = np.asarray(bias)

    nc = _get_nc()
    in_maps = _make_in_maps(x, qweight, qzeros, scales, bias)

    kwargs = {}
    if os.environ.get("AWQ_PROFILE"):
        _enable_profiling()
        kwargs = dict(trace=True, tmpdir=os.environ.get("AWQ_TRACE_DIR") or None)
    res = run_bass_kernel_spmd(nc, in_maps, list(range(N_CORES)), **kwargs)
    LAST_EXEC_NS = res.exec_time_ns

    outT = np.concatenate([res.results[c]["outT"] for c in range(N_CORES)], axis=0)
    return np.ascontiguousarray(outT.T)


def _enable_profiling():
    """Register the NTFF profile hook missing from this image's antenv."""
    import types

    if "antenv.axon_hooks" not in sys.modules:
        import antenv

        mod = types.ModuleType("antenv.axon_hooks")
        mod._hook = None
        mod.set_axon_ntff_profile_hook = lambda h: setattr(mod, "_hook", h)
        mod.get_axon_ntff_profile_hook = lambda: mod._hook
        sys.modules["antenv.axon_hooks"] = mod
        antenv.axon_hooks = mod
        try:
            from trn_agent_boot.trn_boot import _ntff_profile_via_ctypes

            mod.set_axon_ntff_profile_hook(
                _ntff_profile_via_ctypes("/opt/axon/libaxon_pjrt.so")
            )
        except Exception:
            pass
    import concourse.bass_utils as _bu

    _bu.upload_artifacts = lambda tmpdir: "local://skipped"
